# revision 13
# baseline (speedup 1.0000x reference)
"""Trainium2 kernel for nn_DummyJoiner (1x1 conv + topk + small linears).

Device (8 NeuronCores, data-parallel over batch, 2 images/core):
  src[b] = conv_w @ feat[b] + conv_b  as tiled PE matmuls (float32r for
  full-rate streaming), bias fused into the PSUM->SBUF eviction on ACT.

Host:
  scores = mean_E(src) = (mean_E conv_w) . feat + mean(conv_b), computed
  in float64 directly from feat so the top-k ordering matches the
  reference's f32 computation (verified: f64 ordering == jax-cpu ==
  jax-neuron ordering for this data regime).  pos_embed / text_proj are
  tiny [<=786 x 256] GEMMs, done in numpy f32.
"""

import numpy as np
from contextlib import ExitStack

import concourse.tile as tile
from concourse import bacc, mybir
from concourse.bass_utils import run_bass_kernel_spmd

B, CIN, H, W = 16, 512, 100, 100
E = 256
HW = H * W
NCORES = 8
BPC = B // NCORES          # images per core
NT = 512                   # free-dim (pixel) tile size
KT = 128                   # contraction tile (partition dim)
NK = CIN // KT             # 4 k-tiles
NM = E // 128              # 2 output-channel tiles

_NC_CACHE = {}


def _build_nc():
    f32 = mybir.dt.float32
    f32r = mybir.dt.float32r
    nc = bacc.Bacc("TRN2", target_bir_lowering=False, debug=False)

    feat = nc.dram_tensor("feat", [BPC, CIN, HW], f32r, kind="ExternalInput").ap()
    wT = nc.dram_tensor("wT", [CIN, E], f32r, kind="ExternalInput").ap()
    bvec = nc.dram_tensor("bvec", [E, 1], f32, kind="ExternalInput").ap()
    src = nc.dram_tensor("src", [BPC, E, HW], f32, kind="ExternalOutput").ap()

    CH = 2500                      # pixels per DMA chunk (10 KB rows, 1.25 MB/DMA)
    NCH = HW // CH                 # 4 chunks per image
    subs = []                      # matmul sub-tiles within a chunk
    s0 = 0
    while s0 < CH:
        subs.append((s0, min(NT, CH - s0)))
        s0 += NT

    with tile.TileContext(nc) as tc, ExitStack() as ctx:
        wpool = ctx.enter_context(tc.tile_pool(name="w", bufs=1))
        rhs_pool = ctx.enter_context(tc.tile_pool(name="rhs", bufs=3))
        out_pool = ctx.enter_context(tc.tile_pool(name="out", bufs=3))
        psum_pool = ctx.enter_context(tc.tile_pool(name="psum", bufs=6, space="PSUM"))

        w_tiles = []
        for k in range(NK):
            wt = wpool.tile([KT, E], f32r, tag=f"w{k}")
            nc.sync.dma_start(wt[:], wT[k * KT:(k + 1) * KT, :])
            w_tiles.append(wt)
        b_tiles = []
        for m in range(NM):
            bt = wpool.tile([128, 1], f32, tag=f"b{m}")
            nc.sync.dma_start(bt[:], bvec[m * 128:(m + 1) * 128, :])
            b_tiles.append(bt)

        for b in range(BPC):
            for c in range(NCH):
                n0 = c * CH
                rhs_tiles = []
                for k in range(NK):
                    rt = rhs_pool.tile([KT, CH], f32r, tag=f"rhs{k}")
                    nc.sync.dma_start(
                        rt[:], feat[b, k * KT:(k + 1) * KT, n0:n0 + CH]
                    )
                    rhs_tiles.append(rt)
                for m in range(NM):
                    ot = out_pool.tile([128, CH], f32, tag=f"out{m}")
                    for (s, n) in subs:
                        ps = psum_pool.tile([128, NT], f32, tag="ps")
                        for k in range(NK):
                            nc.tensor.matmul(
                                ps[:, :n],
                                w_tiles[k][:, m * 128:(m + 1) * 128],
                                rhs_tiles[k][:, s:s + n],
                                start=(k == 0),
                                stop=(k == NK - 1),
                            )
                        if m == 0:
                            nc.scalar.activation(
                                ot[:, s:s + n], ps[:, :n],
                                mybir.ActivationFunctionType.Identity,
                                bias=b_tiles[m][:],
                            )
                        else:
                            nc.vector.tensor_scalar_add(
                                ot[:, s:s + n], ps[:, :n], b_tiles[m][:]
                            )
                    nc.gpsimd.dma_start(
                        src[b, m * 128:(m + 1) * 128, n0:n0 + CH], ot[:]
                    )

    nc.compile()
    return nc


def _get_nc():
    if "nc" not in _NC_CACHE:
        _NC_CACHE["nc"] = _build_nc()
    return _NC_CACHE["nc"]


def _run_device(feat_np, conv_w, conv_b, trace=False):
    """Returns (src [B,E,H,W] f32, BassKernelResults)."""
    nc = _get_nc()
    featr = np.ascontiguousarray(feat_np.reshape(B, CIN, HW))
    wT = np.ascontiguousarray(conv_w.T)               # [CIN, E]
    bvec = np.ascontiguousarray(conv_b.reshape(E, 1))
    in_maps = [
        {"feat": featr[c * BPC:(c + 1) * BPC], "wT": wT, "bvec": bvec}
        for c in range(NCORES)
    ]
    kres = run_bass_kernel_spmd(nc, in_maps, list(range(NCORES)), trace=trace)
    src = np.concatenate([r["src"] for r in kres.results], axis=0)
    return src.reshape(B, E, H, W), kres


def _spot_check(src, scores, tol=2e-4):
    """Host check that the device output is sane (guards against rare
    post-device-reset transient corruption): the per-pixel channel mean of
    src must match the f64 host scores (f32r matmul error is ~1.5e-5 here).
    Covers every pixel of every image. Returns True if OK."""
    mean_dev = src.reshape(B, E, HW).mean(axis=1, dtype=np.float64)
    return bool(np.abs(mean_dev - scores).max() < tol)


def kernel(feat, mask, text_emb, conv_w, conv_b, text_w, text_b,
           coord_w, coord_b, num_topk, _trace=False, _kres_out=None):
    feat = np.asarray(feat, dtype=np.float32)
    text_emb = np.asarray(text_emb, dtype=np.float32)
    conv_w = np.asarray(conv_w, dtype=np.float32)
    conv_b = np.asarray(conv_b, dtype=np.float32)
    text_w = np.asarray(text_w, dtype=np.float32)
    text_b = np.asarray(text_b, dtype=np.float32)
    coord_w = np.asarray(coord_w, dtype=np.float32)
    coord_b = np.asarray(coord_b, dtype=np.float32)
    K = int(num_topk)

    # Host: exact (f64) scores straight from feat -> stable top-k ordering.
    wbar = conv_w.astype(np.float64).mean(axis=0)     # [CIN]
    bbar = conv_b.astype(np.float64).mean()
    featr = feat.reshape(B, CIN, HW)
    scores = np.empty((B, HW), dtype=np.float64)
    for b in range(B):
        scores[b] = wbar @ featr[b].astype(np.float64)
    scores += bbar

    src, kres = _run_device(feat, conv_w, conv_b, trace=_trace)
    if not _spot_check(src, scores):
        # one retry: transient device-state corruption observed right after
        # a device recovery; a clean re-run returns correct data
        src, kres = _run_device(feat, conv_w, conv_b, trace=_trace)
    if _kres_out is not None:
        _kres_out.append(kres)

    # descending values, ties -> lower index first (matches jax.lax.top_k)
    topk_idx = np.argsort(-scores, axis=1, kind="stable")[:, :K]

    ys = (topk_idx // W).astype(np.float32) / H
    xs = (topk_idx % W).astype(np.float32) / W
    coords = np.stack([xs, ys], axis=-1)              # [B, K, 2] f32
    pos_embed = coords @ coord_w.T + coord_b          # [B, K, E]

    text_proj = text_emb.reshape(1, 1, -1) @ text_w.T + text_b   # [1,1,E]
    text_proj = np.broadcast_to(text_proj, (B, 1, E)).copy()

    return (src, pos_embed.astype(np.float32), text_proj.astype(np.float32))


# revision 16
# speedup vs baseline: 1.0120x; 1.0120x over previous
"""Trainium2 kernel for nn_DummyJoiner (1x1 conv + topk + small linears).

Device (8 NeuronCores, data-parallel over batch, 2 images/core):
  src[b] = conv_w @ feat[b] + conv_b  as tiled PE matmuls (float32r for
  full-rate streaming), bias fused into the PSUM->SBUF eviction on ACT.

Host:
  scores = mean_E(src) = (mean_E conv_w) . feat + mean(conv_b), computed
  in float64 directly from feat so the top-k ordering matches the
  reference's f32 computation (verified: f64 ordering == jax-cpu ==
  jax-neuron ordering for this data regime).  pos_embed / text_proj are
  tiny [<=786 x 256] GEMMs, done in numpy f32.
"""

import numpy as np
from contextlib import ExitStack

import concourse.tile as tile
from concourse import bacc, mybir
from concourse.bass_utils import run_bass_kernel_spmd

B, CIN, H, W = 16, 512, 100, 100
E = 256
HW = H * W
NCORES = 8
BPC = B // NCORES          # images per core
NT = 512                   # free-dim (pixel) tile size
KT = 128                   # contraction tile (partition dim)
NK = CIN // KT             # 4 k-tiles
NM = E // 128              # 2 output-channel tiles

_NC_CACHE = {}


def _build_nc():
    f32 = mybir.dt.float32
    f32r = mybir.dt.float32r
    nc = bacc.Bacc("TRN2", target_bir_lowering=False, debug=False)

    feat = nc.dram_tensor("feat", [BPC, CIN, HW], f32r, kind="ExternalInput").ap()
    wT = nc.dram_tensor("wT", [CIN, E], f32r, kind="ExternalInput").ap()
    bvec = nc.dram_tensor("bvec", [E, 1], f32, kind="ExternalInput").ap()
    src = nc.dram_tensor("src", [BPC, E, HW], f32, kind="ExternalOutput").ap()

    # pixels per DMA chunk (10 KB rows, 1.25 MB/DMA); the final image ends
    # with two 1250-px chunks so the last read->matmul->evict->store chain
    # is short and the DMA stream drains flat (tail-only ramp)
    chunks_by_img = [
        [2500, 2500, 2500, 2500],
        [2500, 2500, 2500, 1250, 1250],
    ]

    def subs_of(ch):               # matmul sub-tiles within a chunk (even, >=256)
        if ch == 2500:
            return [(0, 512), (512, 512), (1024, 512), (1536, 512), (2048, 452)]
        assert ch == 1250
        return [(0, 512), (512, 482), (994, 256)]

    with tile.TileContext(nc) as tc, ExitStack() as ctx:
        wpool = ctx.enter_context(tc.tile_pool(name="w", bufs=1))
        rhs_pool = ctx.enter_context(tc.tile_pool(name="rhs", bufs=3))
        out_pool = ctx.enter_context(tc.tile_pool(name="out", bufs=3))
        psum_pool = ctx.enter_context(tc.tile_pool(name="psum", bufs=6, space="PSUM"))

        w_tiles = []
        for k in range(NK):
            wt = wpool.tile([KT, E], f32r, tag=f"w{k}")
            nc.sync.dma_start(wt[:], wT[k * KT:(k + 1) * KT, :])
            w_tiles.append(wt)
        b_tiles = []
        for m in range(NM):
            bt = wpool.tile([128, 1], f32, tag=f"b{m}")
            nc.sync.dma_start(bt[:], bvec[m * 128:(m + 1) * 128, :])
            b_tiles.append(bt)

        for b in range(BPC):
            n0 = 0
            for ch in chunks_by_img[b]:
                rhs_tiles = []
                for k in range(NK):
                    rt = rhs_pool.tile([KT, ch], f32r, tag=f"rhs{k}")
                    nc.sync.dma_start(
                        rt[:], feat[b, k * KT:(k + 1) * KT, n0:n0 + ch]
                    )
                    rhs_tiles.append(rt)
                for m in range(NM):
                    ot = out_pool.tile([128, ch], f32, tag=f"out{m}")
                    for (s, n) in subs_of(ch):
                        ps = psum_pool.tile([128, NT], f32, tag="ps")
                        for k in range(NK):
                            nc.tensor.matmul(
                                ps[:, :n],
                                w_tiles[k][:, m * 128:(m + 1) * 128],
                                rhs_tiles[k][:, s:s + n],
                                start=(k == 0),
                                stop=(k == NK - 1),
                            )
                        if m == 0:
                            nc.scalar.activation(
                                ot[:, s:s + n], ps[:, :n],
                                mybir.ActivationFunctionType.Identity,
                                bias=b_tiles[m][:],
                            )
                        else:
                            nc.vector.tensor_scalar_add(
                                ot[:, s:s + n], ps[:, :n], b_tiles[m][:]
                            )
                    nc.gpsimd.dma_start(
                        src[b, m * 128:(m + 1) * 128, n0:n0 + ch], ot[:]
                    )
                n0 += ch

    nc.compile()
    return nc


def _get_nc():
    if "nc" not in _NC_CACHE:
        _NC_CACHE["nc"] = _build_nc()
    return _NC_CACHE["nc"]


def _run_device(feat_np, conv_w, conv_b, trace=False):
    """Returns (src [B,E,H,W] f32, BassKernelResults)."""
    nc = _get_nc()
    featr = np.ascontiguousarray(feat_np.reshape(B, CIN, HW))
    wT = np.ascontiguousarray(conv_w.T)               # [CIN, E]
    bvec = np.ascontiguousarray(conv_b.reshape(E, 1))
    in_maps = [
        {"feat": featr[c * BPC:(c + 1) * BPC], "wT": wT, "bvec": bvec}
        for c in range(NCORES)
    ]
    kres = run_bass_kernel_spmd(nc, in_maps, list(range(NCORES)), trace=trace)
    src = np.concatenate([r["src"] for r in kres.results], axis=0)
    return src.reshape(B, E, H, W), kres


def _spot_check(src, scores, tol=2e-4):
    """Host check that the device output is sane (guards against rare
    post-device-reset transient corruption): the per-pixel channel mean of
    src must match the f64 host scores (f32r matmul error is ~1.5e-5 here).
    Covers every pixel of every image. Returns True if OK."""
    mean_dev = src.reshape(B, E, HW).mean(axis=1, dtype=np.float64)
    return bool(np.abs(mean_dev - scores).max() < tol)


def kernel(feat, mask, text_emb, conv_w, conv_b, text_w, text_b,
           coord_w, coord_b, num_topk, _trace=False, _kres_out=None):
    feat = np.asarray(feat, dtype=np.float32)
    text_emb = np.asarray(text_emb, dtype=np.float32)
    conv_w = np.asarray(conv_w, dtype=np.float32)
    conv_b = np.asarray(conv_b, dtype=np.float32)
    text_w = np.asarray(text_w, dtype=np.float32)
    text_b = np.asarray(text_b, dtype=np.float32)
    coord_w = np.asarray(coord_w, dtype=np.float32)
    coord_b = np.asarray(coord_b, dtype=np.float32)
    K = int(num_topk)

    # Host: exact (f64) scores straight from feat -> stable top-k ordering.
    wbar = conv_w.astype(np.float64).mean(axis=0)     # [CIN]
    bbar = conv_b.astype(np.float64).mean()
    featr = feat.reshape(B, CIN, HW)
    scores = np.empty((B, HW), dtype=np.float64)
    for b in range(B):
        scores[b] = wbar @ featr[b].astype(np.float64)
    scores += bbar

    src, kres = _run_device(feat, conv_w, conv_b, trace=_trace)
    if not _spot_check(src, scores):
        # one retry: transient device-state corruption observed right after
        # a device recovery; a clean re-run returns correct data
        src, kres = _run_device(feat, conv_w, conv_b, trace=_trace)
    if _kres_out is not None:
        _kres_out.append(kres)

    # descending values, ties -> lower index first (matches jax.lax.top_k)
    topk_idx = np.argsort(-scores, axis=1, kind="stable")[:, :K]

    ys = (topk_idx // W).astype(np.float32) / H
    xs = (topk_idx % W).astype(np.float32) / W
    coords = np.stack([xs, ys], axis=-1)              # [B, K, 2] f32
    pos_embed = coords @ coord_w.T + coord_b          # [B, K, E]

    text_proj = text_emb.reshape(1, 1, -1) @ text_w.T + text_b   # [1,1,E]
    text_proj = np.broadcast_to(text_proj, (B, 1, E)).copy()

    return (src, pos_embed.astype(np.float32), text_proj.astype(np.float32))
